# revision 2
# baseline (speedup 1.0000x reference)
"""AWQ quantized linear (nn_AWQLinear) on 8 TRN2 NeuronCores.

  out[b,s,o] = sum_k x[b,s,k] * act_scales[k] * w[o,k] / mean(act_scales)
  w[o,k]     = (qweight[o,g,j] - zeros[o,g]) * scales[o,g],  k = 128*g + j

Strategy (column-parallel): shard qweight/scales/zeros along out_features
across the 8 cores (1376 out-features each); replicate x and act_scales.

Per core, weight prep dequantizes + transposes the shard with TensorE only:
the transpose-matmul's stationary operand is [127 code rows | ones row] and
its moving operand is a host-laid-out [diag(scales); -zeros*scales] tile, so
a single 128-contraction matmul yields W^T = (q - z) * s for one (o-tile, g)
slab in PSUM. The PSUM->SBUF drain folds act_scales/mean (per-partition k)
and is split across DVE and ACT. This removes the per-group ScalarE dequant
pass and its serialization of the prep window.

Everything streams as bf16 (fp16 measured ~1.2x slower per matmul row on
HW). Main loop: out[t,o] += xT.T @ wT with 508/508/360-wide PSUM chunks
aligned to the prep packs, fp32 accumulation. Weight-prep blocks are
interleaved with the first two token tiles' matmul chains so TensorE never
idles while the prep streams in; outputs DMA per chunk from SBUF staging.

Host-side work is limited to sharding/layout (slicing, transposes to
contraction-major, dtype casts, arranging scales diagonally / ones padding)
plus tiny parameter prep (z*s and the scalar mean(act_scales) fold).
"""

import os

# the kernel executes on the axon/neuron jax backend; a cpu-pinned
# JAX_PLATFORMS (some harnesses set it for reference runs) would hide the
# NeuronCores from run_bass_kernel_spmd's PJRT path
if os.environ.get("JAX_PLATFORMS", "").strip() == "cpu":
    del os.environ["JAX_PLATFORMS"]

from contextlib import ExitStack
from itertools import chain as chain_iter

import numpy as np
import ml_dtypes

import concourse.bacc as bacc
import concourse.mybir as mybir
import concourse.tile as tile
from concourse.bass import ts
from concourse import bass_utils

# problem shape (hardcoded per the harness contract)
B, S, IN_F, OUT_F = 4, 2048, 4096, 11008
T = B * S                      # 8192 tokens
K = IN_F                       # 4096 contraction
G = 32                         # quant groups of 128 (== partition count)
NCORES = 8
OS = OUT_F // NCORES           # 1376 out-features per core
TCH = 256                      # token chunk resident in SBUF
NTCH = T // TCH                # 32 chunks
TPW = 127                      # o-rows per transpose tile (128th row = ones)
N_OT = 11                      # o-tiles: 10x127 + 1x106
P_W = [TPW] * 10 + [OS - 10 * TPW]          # widths: 127..127, 106
# packs of o-tiles sharing one PSUM bank during weight prep
PACKS = [(0, [0, 1, 2, 3]), (508, [4, 5, 6, 7]), (1016, [8, 9, 10])]
OC_CHUNKS = [(0, 508), (508, 508), (1016, 360)]  # pack-aligned psum chunks

BF16 = mybir.dt.bfloat16
F32 = mybir.dt.float32

_CACHE = {}


def _build():
    """Emit the per-core Tile program (identical on all 8 cores)."""
    nc = bacc.Bacc("TRN2", target_bir_lowering=False, debug=False)
    xp_d = nc.dram_tensor("xp", [NTCH, 128, G, TCH], BF16, kind="ExternalInput").ap()
    # padded code tiles: rows 0..126 = qweight codes, row 127 = 1.0
    qd_d = nc.dram_tensor("qd", [N_OT, 128, K], BF16, kind="ExternalInput").ap()
    # rows 0..126 = diag(scales) per (o-tile, g), row 127 = -(zeros*scales)
    rx_d = nc.dram_tensor("rx", [N_OT, 128, G, 128], BF16, kind="ExternalInput").ap()
    aT_d = nc.dram_tensor("aT", [128, G], F32, kind="ExternalInput").ap()
    out_d = nc.dram_tensor("out", [T, OS], F32, kind="ExternalOutput").ap()

    with ExitStack() as ctx:
        tc = ctx.enter_context(tile.TileContext(nc))
        const = ctx.enter_context(tc.tile_pool(name="const", bufs=1))
        wres = ctx.enter_context(tc.tile_pool(name="wres", bufs=1))
        qpool = ctx.enter_context(tc.tile_pool(name="qpool", bufs=14))
        rpool = ctx.enter_context(tc.tile_pool(name="rpool", bufs=14))
        xpool = ctx.enter_context(tc.tile_pool(name="xpool", bufs=3))
        # PSUM: 5 banks for the main chains + 3 for weight-prep = 8
        mpsum = ctx.enter_context(tc.tile_pool(name="mpsum", bufs=2, space="PSUM"))
        m2psum = ctx.enter_context(tc.tile_pool(name="m2psum", bufs=2, space="PSUM"))
        ptpsum = ctx.enter_context(tc.tile_pool(name="ptpsum", bufs=2, space="PSUM"))
        opool = ctx.enter_context(tc.tile_pool(name="opool", bufs=2))

        a_sb = const.tile([128, G], F32)  # act_scales/mean, contraction-major
        nc.sync.dma_start(out=a_sb, in_=aT_d)

        # resident dequantized transposed weights: [k%128, g, o] bf16
        wT = wres.tile([128, G, OS], BF16)

        def mm_psum(j, n):
            pool = m2psum if j == 2 else mpsum
            return pool.tile([128, n], F32, tag=f"mm{j}", name=f"ps{j}")

        # ---- weight prep, interleaved with token tiles 0+1's chains ----
        # T-group for (pack, g): <=4 transpose-matmuls into one PSUM bank;
        # drain folds a = act_scales/mean (per-partition), cast to bf16,
        # alternating DVE/ACT. Chunk j of the main loop reads exactly pack
        # j's o-range, so chain members for (chunk=pk, octile=q) become
        # runnable one block after block (q, pk) — the PE stays fed while
        # the prep streams its 23 MB of qd/rx slabs.
        # x chunk 0 ahead of everything (prep's chains need it first)
        xts = {}
        xt0 = xpool.tile([128, G, TCH], BF16, tag="x", name="xt0")
        nc.sync.dma_start(out=xt0, in_=xp_d[0])
        xts[0] = xt0

        # all prep slab DMAs up front, in block order: the pools' 12-deep
        # rings keep ~3 blocks of prefetch in flight so T-groups never wait
        slab_tiles = {}
        for bn in range(12):
            q, pk = divmod(bn, 3)
            for i in PACKS[pk][1]:
                qsl = qpool.tile([128, K // 4], BF16, tag="q", name=f"qsl{i}")
                rsl = rpool.tile([128, 8, 128], BF16, tag="r", name=f"rsl{i}")
                if bn == 0:
                    # split so the first T-groups can start after ~64 KB
                    nc.sync.dma_start(out=qsl[:, :256], in_=qd_d[i][:, :256])
                    nc.sync.dma_start(out=rsl[:, :2, :], in_=rx_d[i][:, :2, :])
                    nc.sync.dma_start(out=qsl[:, 256:], in_=qd_d[i][:, 256:K // 4])
                    nc.sync.dma_start(out=rsl[:, 2:, :], in_=rx_d[i][:, 2:8, :])
                else:
                    nc.sync.dma_start(out=qsl, in_=qd_d[i][:, ts(q, K // 4)])
                    nc.sync.dma_start(out=rsl, in_=rx_d[i][:, ts(q, 8), :])
                slab_tiles[(i, q)] = (qsl, rsl)
        for tci in (1, 2):  # next x chunks behind the slabs, ahead of phase M
            xt_pre = xpool.tile([128, G, TCH], BF16, tag="x", name=f"xt{tci}")
            nc.sync.dma_start(out=xt_pre, in_=xp_d[tci])
            xts[tci] = xt_pre
        ps01 = [[mm_psum(j, n) for j, (_, n) in enumerate(OC_CHUNKS)]
                for _ in range(2)]
        for pst in ps01:
            nc.vector.memset(pst[0], 0.0)
            nc.vector.memset(pst[1], 0.0)
            nc.scalar.activation(out=pst[2], in_=pst[2], func=mybir.ActivationFunctionType.Identity, scale=0.0)

        def unit_mms(tile_i, j, oct_i):
            o0, n = OC_CHUNKS[j]
            for gg in range(8):
                g = 8 * oct_i + gg
                yield (ps01[tile_i][j], xt0[:, g, ts(tile_i, 128)],
                       wT[:, g, o0:o0 + n])

        def emit_m(stream, k):
            for _ in range(k):
                mm = next(stream, None)
                if mm is None:
                    return
                out_ps, lhsT, rhs = mm
                nc.tensor.matmul(out_ps, lhsT=lhsT, rhs=rhs, start=False,
                                 stop=False, skip_group_check=True)

        for bn in range(12):        # blocks: (g-octile q, pack pk)
            q, pk = divmod(bn, 3)
            po, tiles = PACKS[pk]
            slabs = {i: slab_tiles[(i, q)] for i in tiles}
            pw = sum(P_W[i] for i in tiles)
            # ready main-chain members: one (chunk, octile) unit per token
            # tile, runnable since the previous block produced its weights
            if bn >= 1:
                jj, oo = (bn - 1) % 3, (bn - 1) // 3
                mstream = chain_iter(unit_mms(0, jj, oo), unit_mms(1, jj, oo))
            else:
                mstream = iter(())
            for gg in range(8):
                g = 8 * q + gg
                pt = ptpsum.tile([128, 508], F32, name="pt")
                for si, i in enumerate(tiles):
                    nc.tensor.matmul(
                        pt[:, si * TPW:si * TPW + P_W[i]],
                        lhsT=slabs[i][0][:, ts(gg, 128)],
                        rhs=slabs[i][1][:, gg, :P_W[i]],
                        start=(si == 0), stop=(si == len(tiles) - 1),
                    )
                if gg % 2 == 1:
                    nc.scalar.activation(
                        out=wT[:, g, po:po + pw], in_=pt[:, :pw],
                        func=mybir.ActivationFunctionType.Identity,
                        scale=a_sb[:, g:g + 1],
                    )
                else:
                    nc.vector.tensor_scalar_mul(
                        wT[:, g, po:po + pw], pt[:, :pw], a_sb[:, g:g + 1],
                    )
                emit_m(mstream, 2)
            emit_m(mstream, 16)
        for jj, oo in ((2, 3),):    # last unit pair after the final block
            emit_m(chain_iter(unit_mms(0, jj, oo), unit_mms(1, jj, oo)), 16)

        def drain_out(ps, t0):
            for j, (o0, n) in enumerate(OC_CHUNKS):
                ob = opool.tile([128, n], F32, tag=f"ob{j}")
                if j == 2:  # keep DVE and ACT both busy on the drains
                    nc.scalar.activation(
                        out=ob, in_=ps[j],
                        func=mybir.ActivationFunctionType.Identity,
                    )
                else:
                    nc.vector.tensor_copy(ob, ps[j])
                nc.sync.dma_start(out=out_d[t0:t0 + 128, o0:o0 + n], in_=ob)

        drain_out(ps01[0], 0)
        drain_out(ps01[1], 128)

        # ---- main loop: out[t,o] = sum_g xT[:,g,t].T @ wT[:,g,o] ----
        for tci in range(NTCH):
            xt = xts.get(tci)
            if xt is None:
                xt = xpool.tile([128, G, TCH], BF16, tag="x", name="xt")
                nc.sync.dma_start(out=xt, in_=xp_d[tci])
            for tt in range(TCH // 128):
                if tci == 0:
                    continue  # both 128-token tiles handled during prep
                ps = [mm_psum(j, n) for j, (_, n) in enumerate(OC_CHUNKS)]
                nc.vector.memset(ps[0], 0.0)
                nc.vector.memset(ps[1], 0.0)
                nc.scalar.activation(out=ps[2], in_=ps[2], func=mybir.ActivationFunctionType.Identity, scale=0.0)
                for g in range(G):
                    lhsT = xt[:, g, ts(tt, 128)]
                    for j, (o0, n) in enumerate(OC_CHUNKS):
                        nc.tensor.matmul(
                            ps[j], lhsT=lhsT, rhs=wT[:, g, o0:o0 + n],
                            start=False, stop=False, skip_group_check=True,
                        )
                drain_out(ps, tci * TCH + tt * 128)
    nc.compile()
    return nc


def _get_program():
    if "nc" not in _CACHE:
        _CACHE["nc"] = _build()
    return _CACHE["nc"]


def _host_prep(x, qweight, scales, zeros, act_scales):
    """Shard + layout prep: contraction-major repacks, dtype casts, the
    padded code tiles and the diag(s)/-z*s transpose operand tiles."""
    xp = np.ascontiguousarray(
        x.reshape(NTCH, TCH, G, 128).transpose(0, 3, 2, 1)
    ).astype(ml_dtypes.bfloat16)                               # [NTCH,128,G,TCH]
    qflat = qweight.reshape(OUT_F, K)
    a_vec = (act_scales / act_scales.mean()).astype(np.float32)
    aT = np.ascontiguousarray(a_vec.reshape(G, 128).T)  # [128, G]

    in_maps = []
    for c in range(NCORES):
        o0c = c * OS
        qs = qflat[o0c:o0c + OS]
        sc = scales[o0c:o0c + OS]
        zr = zeros[o0c:o0c + OS]
        qd = np.zeros((N_OT, 128, K), dtype=ml_dtypes.bfloat16)
        rx = np.zeros((N_OT, 128, G, 128), dtype=ml_dtypes.bfloat16)
        nzs = (-(zr * sc)).astype(np.float32)           # [OS, G]
        for i in range(N_OT):
            p = P_W[i]
            rows = np.arange(p)
            qd[i, :p] = qs[i * TPW:i * TPW + p]
            qd[i, 127] = 1.0
            rx[i, rows, :, rows] = sc[i * TPW:i * TPW + p]
            rx[i, 127, :, :p] = nzs[i * TPW:i * TPW + p].T
        in_maps.append({"xp": xp, "qd": qd, "rx": rx, "aT": aT})
    return in_maps


def kernel(x, qweight, scales, zeros, act_scales):
    x = np.asarray(x, dtype=np.float32)
    qweight = np.asarray(qweight)
    scales = np.asarray(scales, dtype=np.float32)
    zeros = np.asarray(zeros, dtype=np.float32)
    act_scales = np.asarray(act_scales, dtype=np.float32)

    in_maps = _host_prep(x, qweight, scales, zeros, act_scales)

    nc = _get_program()
    trace = bool(os.environ.get("KERNEL_TRACE"))
    if trace:
        try:  # register the NTFF profile hook if the image's antenv lacks it
            from antenv.axon_hooks import get_axon_ntff_profile_hook  # noqa: F401
        except ImportError:
            import sys, types, antenv  # noqa: PLC0415
            mod = types.ModuleType("antenv.axon_hooks")
            _h = [None]
            mod.set_axon_ntff_profile_hook = lambda h: _h.__setitem__(0, _h[0] or h)
            mod.get_axon_ntff_profile_hook = lambda: _h[0]
            sys.modules["antenv.axon_hooks"] = mod
            antenv.axon_hooks = mod
            from trn_agent_boot.trn_boot import _ntff_profile_via_ctypes
            mod.set_axon_ntff_profile_hook(
                _ntff_profile_via_ctypes("/opt/axon/libaxon_pjrt.so")
            )
    res = bass_utils.run_bass_kernel_spmd(
        nc, in_maps, core_ids=list(range(NCORES)), trace=trace
    )
    kernel.last_exec_time_ns = res.exec_time_ns
    kernel.last_result = res
    if trace and res.exec_time_ns is not None:
        print(f"HW exec time: {res.exec_time_ns} ns")

    out = np.concatenate([res.results[c]["out"] for c in range(NCORES)], axis=1)
    return np.ascontiguousarray(out.reshape(B, S, OUT_F))


kernel.last_exec_time_ns = None

